# revision 2
# baseline (speedup 1.0000x reference)
"""BinarizeLinear kernel for TRN2: out = x @ sign(W).

x: [32768, 512] f32, W: [512, 512] f32 -> out: [32768, 512] f32.

Data-parallel across 8 NeuronCores: each core handles 4096 tokens, W is
replicated. Per core:
  - sign(W) computed on ScalarE (ACT Sign), cast to bf16 (exact for +-1).
  - x tiles loaded naturally [128 tok, 512 din], transposed on TensorE
    (PE contracts over the partition dim, so x must present d_in on
    partitions), cast to bf16 during the PSUM->SBUF copy.
  - bf16 matmuls (1 cyc/row vs 4 for f32) accumulate [128 tok, 512 dout]
    f32 tiles in PSUM; copied to SBUF and stored in 1 MiB DMA batches.
"""

import sys

if "/opt/trn_rl_repo" not in sys.path:
    sys.path.insert(0, "/opt/trn_rl_repo")

import json

import numpy as np

import concourse.bass as bass
import concourse.mybir as mybir
import concourse.tile as tile
from concourse.bass import ds
from concourse.masks import make_identity

# ---------------------------------------------------------------------------
# Workaround: the pinned walrus only accepts ONE sync wait and ONE sync
# update per instruction ("Too many sync wait commands" in setupSyncWait),
# but Tile's kernel-tail Drain carries one wait per outstanding semaphore.
# Split extras onto single-wait NoOps before (waits) / after (updates) the
# instruction — same engine, so program order preserves the semantics.
# ---------------------------------------------------------------------------

_split_uid = 0


def _split_sync(bir_json: bytes) -> bytes:
    global _split_uid
    bir = json.loads(bir_json)
    changed = False
    for fn in bir.get("functions", []):
        for blk in fn.get("blocks", []):
            insts = blk.get("instructions", [])
            out = []
            for inst in insts:
                si = inst.get("sync_info") or {}
                waits = si.get("on_wait") or []
                updates = si.get("on_update") or []
                if len(waits) > 1:
                    for w in waits[:-1]:
                        _split_uid += 1
                        out.append(
                            {
                                "name": f"I-syncsplit-w{_split_uid}",
                                "engine": inst["engine"],
                                "opcode": "NoOp",
                                "ins": [],
                                "outs": [],
                                "sync_info": {"on_update": [], "on_wait": [w]},
                            }
                        )
                    si["on_wait"] = [waits[-1]]
                    changed = True
                out.append(inst)
                if len(updates) > 1:
                    si["on_update"] = [updates[0]]
                    for u in updates[1:]:
                        _split_uid += 1
                        out.append(
                            {
                                "name": f"I-syncsplit-u{_split_uid}",
                                "engine": inst["engine"],
                                "opcode": "NoOp",
                                "ins": [],
                                "outs": [],
                                "sync_info": {"on_update": [u], "on_wait": []},
                            }
                        )
                    changed = True
            blk["instructions"] = out
    if not changed:
        return bir_json
    return json.dumps(bir).encode()


def _install_sync_split_patch() -> None:
    import concourse.bass2jax as bass2jax
    import concourse.bass_utils as bass_utils

    orig = bass_utils.compile_bir_kernel
    if getattr(orig, "_sync_split_patched", False):
        return

    def patched(bir_json, tmpdir, neff_name="file.neff", **kw):
        return orig(_split_sync(bir_json), tmpdir, neff_name, **kw)

    patched._sync_split_patched = True
    bass_utils.compile_bir_kernel = patched
    bass2jax.compile_bir_kernel = patched


_install_sync_split_patch()

N_CORES = 8
N_TOKENS = 32768
D_IN = 512
D_OUT = 512

TOK_PER_CORE = N_TOKENS // N_CORES  # 4096
P = 128  # partitions
K_CHUNKS = D_IN // P  # 4
MACRO = 4  # token tiles per DMA batch (4 * 128 * 512 * 4B = 1 MiB)
N_MACRO = TOK_PER_CORE // (MACRO * P)  # 8

F32 = mybir.dt.float32
BF16 = mybir.dt.bfloat16


def build_kernel(nc: bass.Bass) -> None:
    x = nc.dram_tensor("x", [TOK_PER_CORE, D_IN], F32, kind="ExternalInput").ap()
    w = nc.dram_tensor("W", [D_IN, D_OUT], F32, kind="ExternalInput").ap()
    out = nc.dram_tensor("out", [TOK_PER_CORE, D_OUT], F32, kind="ExternalOutput").ap()

    # [p, a, d] view: token t = a*128 + p within a macro block of 512 tokens
    x_v = x.rearrange("(a p) d -> p a d", p=P)  # [128, 32, 512]
    out_v = out.rearrange("(a p) d -> p a d", p=P)  # [128, 32, 512]
    w_v = w.rearrange("(k p) d -> p k d", p=P)  # [128, 4, 512]

    with tile.TileContext(nc) as tc:
        with (
            tc.tile_pool(name="const", bufs=1) as const_pool,
            tc.tile_pool(name="xin", bufs=3) as xin_pool,
            tc.tile_pool(name="xt", bufs=4) as xt_pool,
            tc.tile_pool(name="outsb", bufs=3) as out_pool,
            tc.tile_pool(name="xt_ps", bufs=3, space="PSUM") as xtps_pool,
            tc.tile_pool(name="out_ps", bufs=3, space="PSUM") as outps_pool,
        ):
            # --- constants: identity for PE transpose, binarized weight ---
            ident = const_pool.tile([P, P], F32)
            make_identity(nc, ident[:])

            w_f32 = const_pool.tile([P, K_CHUNKS, D_OUT], F32)
            nc.sync.dma_start(w_f32[:], w_v[:])
            w_b = const_pool.tile([P, K_CHUNKS, D_OUT], BF16)
            for k in range(K_CHUNKS):
                # sign(w): ACT LUT; +-1/0 are exact in bf16
                nc.scalar.activation(
                    w_b[:, k, :], w_f32[:, k, :], mybir.ActivationFunctionType.Sign
                )

            # --- main loop: 8 macro blocks of 512 tokens ---
            for j in range(N_MACRO):
                xin = xin_pool.tile([P, MACRO, D_IN], F32)
                nc.sync.dma_start(xin[:], x_v[:, ds(j * MACRO, MACRO), :])

                out_sb = out_pool.tile([P, MACRO, D_OUT], F32)

                for a in range(MACRO):
                    # transpose [128 tok, 512 din] -> 4x [128 din, 128 tok]
                    xt_ps = xtps_pool.tile([P, D_IN], F32)
                    for k in range(K_CHUNKS):
                        nc.tensor.transpose(
                            xt_ps[:, ds(k * P, P)],
                            xin[:, a, ds(k * P, P)],
                            ident[:],
                        )
                    xt_sb = xt_pool.tile([P, D_IN], BF16)
                    nc.any.tensor_copy(xt_sb[:], xt_ps[:])

                    out_ps = outps_pool.tile([P, D_OUT], F32)
                    for k in range(K_CHUNKS):
                        nc.tensor.matmul(
                            out_ps[:],
                            xt_sb[:, ds(k * P, P)],
                            w_b[:, k, :],
                            start=(k == 0),
                            stop=(k == K_CHUNKS - 1),
                        )
                    nc.any.tensor_copy(out_sb[:, a, :], out_ps[:])

                nc.sync.dma_start(out_v[:, ds(j * MACRO, MACRO), :], out_sb[:])


def _build_nc() -> bass.Bass:
    nc = bass.Bass(
        "TRN2",
        target_bir_lowering=False,
        debug=False,
        num_devices=N_CORES,
    )
    build_kernel(nc)
    return nc


_NC_CACHE = None


def kernel(**inputs: np.ndarray) -> np.ndarray:
    from concourse.bass_utils import run_bass_kernel_spmd

    global _NC_CACHE
    x = np.ascontiguousarray(inputs["x"], dtype=np.float32)
    w = np.ascontiguousarray(inputs["W"], dtype=np.float32)
    assert x.shape == (N_TOKENS, D_IN) and w.shape == (D_IN, D_OUT)

    if _NC_CACHE is None:
        _NC_CACHE = _build_nc()
    nc = _NC_CACHE

    shards = np.split(x, N_CORES, axis=0)
    in_maps = [{"x": s, "W": w} for s in shards]
    res = run_bass_kernel_spmd(nc, in_maps, list(range(N_CORES)))
    return np.concatenate([res.results[i]["out"] for i in range(N_CORES)], axis=0)


if __name__ == "__main__":
    rng = np.random.default_rng(0)
    x = rng.standard_normal((N_TOKENS, D_IN), dtype=np.float32)
    w = rng.standard_normal((D_IN, D_OUT), dtype=np.float32)
    got = kernel(x=x, W=w)
    want = x @ np.sign(w)
    err = np.linalg.norm(got - want) / np.linalg.norm(want)
    print("rel err:", err)


# revision 5
# speedup vs baseline: 140.3015x; 140.3015x over previous
"""BinarizeLinear kernel for TRN2: out = x @ sign(W).

x: [32768, 512] f32, W: [512, 512] f32 -> out: [32768, 512] f32.

Data-parallel across 8 NeuronCores: each core handles 4096 tokens, W is
replicated. Per core:
  - sign(W) computed on ScalarE (ACT Sign), cast to bf16 (exact for +-1).
  - x tiles loaded naturally [128 tok, 512 din], transposed on TensorE
    (PE contracts over the partition dim, so x must present d_in on
    partitions), cast to bf16 during the PSUM->SBUF copy.
  - bf16 matmuls (1 cyc/row vs 4 for f32) accumulate [128 tok, 512 dout]
    f32 tiles in PSUM; copied to SBUF and stored in 1 MiB DMA batches.
"""

import sys

if "/opt/trn_rl_repo" not in sys.path:
    sys.path.insert(0, "/opt/trn_rl_repo")

import json

import numpy as np

import concourse.bass as bass
import concourse.mybir as mybir
import concourse.tile as tile
from concourse.bass import ds
from concourse.masks import make_identity

# ---------------------------------------------------------------------------
# Workaround: the pinned walrus only accepts ONE sync wait and ONE sync
# update per instruction ("Too many sync wait commands" in setupSyncWait),
# but Tile's kernel-tail Drain carries one wait per outstanding semaphore.
# Split extras onto single-wait NoOps before (waits) / after (updates) the
# instruction — same engine, so program order preserves the semantics.
# ---------------------------------------------------------------------------

_split_uid = 0


def _split_sync(bir_json: bytes) -> bytes:
    global _split_uid
    bir = json.loads(bir_json)
    changed = False
    for fn in bir.get("functions", []):
        for blk in fn.get("blocks", []):
            insts = blk.get("instructions", [])
            out = []
            for inst in insts:
                si = inst.get("sync_info") or {}
                waits = si.get("on_wait") or []
                updates = si.get("on_update") or []
                if len(waits) > 1:
                    for w in waits[:-1]:
                        _split_uid += 1
                        out.append(
                            {
                                "name": f"I-syncsplit-w{_split_uid}",
                                "engine": inst["engine"],
                                "opcode": "NoOp",
                                "ins": [],
                                "outs": [],
                                "sync_info": {"on_update": [], "on_wait": [w]},
                            }
                        )
                    si["on_wait"] = [waits[-1]]
                    changed = True
                out.append(inst)
                if len(updates) > 1:
                    si["on_update"] = [updates[0]]
                    for u in updates[1:]:
                        _split_uid += 1
                        out.append(
                            {
                                "name": f"I-syncsplit-u{_split_uid}",
                                "engine": inst["engine"],
                                "opcode": "NoOp",
                                "ins": [],
                                "outs": [],
                                "sync_info": {"on_update": [u], "on_wait": []},
                            }
                        )
                    changed = True
            blk["instructions"] = out
    if not changed:
        return bir_json
    return json.dumps(bir).encode()


def _install_sync_split_patch() -> None:
    import concourse.bass2jax as bass2jax
    import concourse.bass_utils as bass_utils

    orig = bass_utils.compile_bir_kernel
    if getattr(orig, "_sync_split_patched", False):
        return

    def patched(bir_json, tmpdir, neff_name="file.neff", **kw):
        return orig(_split_sync(bir_json), tmpdir, neff_name, **kw)

    patched._sync_split_patched = True
    bass_utils.compile_bir_kernel = patched
    bass2jax.compile_bir_kernel = patched


_install_sync_split_patch()

N_CORES = 8
N_TOKENS = 32768
D_IN = 512
D_OUT = 512

TOK_PER_CORE = N_TOKENS // N_CORES  # 4096
P = 128  # partitions
K_CHUNKS = D_IN // P  # 4
MACRO = 4  # token tiles per DMA batch (4 * 128 * 512 * 4B = 1 MiB)
N_MACRO = TOK_PER_CORE // (MACRO * P)  # 8

F32 = mybir.dt.float32
BF16 = mybir.dt.bfloat16


def build_kernel(nc: bass.Bass, repeat: int = 1) -> None:
    x = nc.dram_tensor("x", [TOK_PER_CORE, D_IN], F32, kind="ExternalInput").ap()
    w = nc.dram_tensor("W", [D_IN, D_OUT], F32, kind="ExternalInput").ap()
    out = nc.dram_tensor("out", [TOK_PER_CORE, D_OUT], F32, kind="ExternalOutput").ap()

    # [p, a, d] view: token t = a*128 + p within a macro block of 512 tokens
    x_v = x.rearrange("(a p) d -> p a d", p=P)  # [128, 32, 512]
    out_v = out.rearrange("(a p) d -> p a d", p=P)  # [128, 32, 512]
    w_v = w.rearrange("(k p) d -> p k d", p=P)  # [128, 4, 512]

    with tile.TileContext(nc) as tc:
        with (
            tc.tile_pool(name="const", bufs=1) as const_pool,
            tc.tile_pool(name="xin", bufs=3) as xin_pool,
            tc.tile_pool(name="xt", bufs=4) as xt_pool,
            tc.tile_pool(name="outsb", bufs=3) as out_pool,
            tc.tile_pool(name="xt_ps", bufs=3, space="PSUM") as xtps_pool,
            tc.tile_pool(name="out_ps", bufs=3, space="PSUM") as outps_pool,
        ):
            # --- constants: identity for PE transpose, binarized weight ---
            ident = const_pool.tile([P, P], F32)
            make_identity(nc, ident[:])

            w_f32 = const_pool.tile([P, K_CHUNKS, D_OUT], F32)
            nc.sync.dma_start(w_f32[:], w_v[:])
            w_b = const_pool.tile([P, K_CHUNKS, D_OUT], BF16)
            for k in range(K_CHUNKS):
                # sign(w): ACT LUT; +-1/0 are exact in bf16
                nc.scalar.activation(
                    w_b[:, k, :], w_f32[:, k, :], mybir.ActivationFunctionType.Sign
                )

            # --- main loop: 8 macro blocks of 512 tokens ---
            for j in [jj for _ in range(repeat) for jj in range(N_MACRO)]:
                xin = xin_pool.tile([P, MACRO, D_IN], F32)
                nc.sync.dma_start(xin[:], x_v[:, ds(j * MACRO, MACRO), :])

                out_sb = out_pool.tile([P, MACRO, D_OUT], F32)

                for a in range(MACRO):
                    # transpose [128 tok, 512 din] -> 4x [128 din, 128 tok]
                    xt_ps = xtps_pool.tile([P, D_IN], F32)
                    for k in range(K_CHUNKS):
                        nc.tensor.transpose(
                            xt_ps[:, ds(k * P, P)],
                            xin[:, a, ds(k * P, P)],
                            ident[:],
                        )
                    xt_sb = xt_pool.tile([P, D_IN], BF16)
                    nc.any.tensor_copy(xt_sb[:], xt_ps[:])

                    out_ps = outps_pool.tile([P, D_OUT], F32)
                    for k in range(K_CHUNKS):
                        nc.tensor.matmul(
                            out_ps[:],
                            xt_sb[:, ds(k * P, P)],
                            w_b[:, k, :],
                            start=(k == 0),
                            stop=(k == K_CHUNKS - 1),
                        )
                    nc.any.tensor_copy(out_sb[:, a, :], out_ps[:])

                nc.sync.dma_start(out_v[:, ds(j * MACRO, MACRO), :], out_sb[:])


def _build_nc(repeat: int = 1) -> bass.Bass:
    nc = bass.Bass(
        "TRN2",
        target_bir_lowering=False,
        debug=False,
        num_devices=N_CORES,
    )
    build_kernel(nc, repeat=repeat)
    return nc


_NC_CACHE = None


def kernel(**inputs: np.ndarray) -> np.ndarray:
    from concourse.bass_utils import run_bass_kernel_spmd

    global _NC_CACHE
    x = np.ascontiguousarray(inputs["x"], dtype=np.float32)
    w = np.ascontiguousarray(inputs["W"], dtype=np.float32)
    assert x.shape == (N_TOKENS, D_IN) and w.shape == (D_IN, D_OUT)

    if _NC_CACHE is None:
        _NC_CACHE = _build_nc()
    nc = _NC_CACHE

    shards = np.split(x, N_CORES, axis=0)
    in_maps = [{"x": s, "W": w} for s in shards]
    res = run_bass_kernel_spmd(nc, in_maps, list(range(N_CORES)))
    return np.concatenate([res.results[i]["out"] for i in range(N_CORES)], axis=0)


if __name__ == "__main__":
    rng = np.random.default_rng(0)
    x = rng.standard_normal((N_TOKENS, D_IN), dtype=np.float32)
    w = rng.standard_normal((D_IN, D_OUT), dtype=np.float32)
    got = kernel(x=x, W=w)
    want = x @ np.sign(w)
    err = np.linalg.norm(got - want) / np.linalg.norm(want)
    print("rel err:", err)
